# revision 5
# baseline (speedup 1.0000x reference)
"""WaveNet-style dilated conv stack on 8 TRN2 NeuronCores.

Sharding: batch(4) x time(2). Each core gets one batch element and one
4096-sample half of the sequence plus a 2048-sample left halo window
(receptive field = 2046). Left-half cores get zeros in the halo plus a
0.0 mask that re-zeroes the halo region after every layer, which exactly
emulates causal zero padding; right-half cores carry real halo data and
mask 1.0 (redundant halo recompute, zero inter-core communication).

Per layer: the dilated conv is 24 accumulated 128x128x512 matmuls per
512-col tile, reading shifted views of a persistent in-SBUF x buffer
(no im2col). Gated tanh*sigmoid on ScalarE with the conv bias fused into
the activation; 1x1 conv = 4 matmuls; bias + residual fused into one
VectorE scalar_tensor_tensor; x updated in place (right-to-left tile
order makes that safe).
"""

import sys
import numpy as np

sys.path.insert(0, "/opt/trn_rl_repo")

DILATIONS = [1, 2, 4, 8, 16, 32, 64, 128, 256, 512]
L = 10
C = 256
B = 4
T = 8192
CH = T // 2          # per-core chunk
U0 = 2048            # halo width (>= receptive field 2046)
NEXT = U0 + CH       # per-core x window
TW = 512             # tile width

# matmul input dtype: "f32r" (fp32 storage, relaxed-precision PE) or "bf16"
MM_MODE = "f32r"

_CACHE = {}


def _h_after(l):
    """Receptive-field halo needed for the output of layer l."""
    return 2 * sum(DILATIONS[l + 1:])


def _layer_tiles(l):
    """(lo, hi) output-column tiles for layer l, process right-to-left."""
    a = U0 - _h_after(l)  # first output col this layer must produce
    edges = [a]
    e = (a // TW + 1) * TW
    while e <= NEXT:
        edges.append(e)
        e += TW
    tiles = [(edges[i], edges[i + 1]) for i in range(len(edges) - 1)]
    return tiles[::-1]  # right to left


def _legalize_waits(nc, keep=1):
    """This walrus build rejects instructions carrying more than ~1 sync
    wait. Move extra waits onto preceding same-engine NOPs, one each."""
    import concourse.mybir as mybir

    n_split = 0
    for f in nc.m.functions:
        for blk in f.blocks:
            out = []
            for inst in blk.instructions:
                si = inst.sync_info
                if si is not None and len(si.on_wait) > keep:
                    waits = list(si.on_wait)
                    for w in waits[:-keep]:
                        nop = mybir.InstNoOp(
                            name=nc.get_next_instruction_name(), ins=[], outs=[]
                        )
                        nop.engine = inst.engine
                        nop.sync_info = mybir.SyncInfo(on_wait=[w], on_update=[])
                        out.append(nop)
                        n_split += 1
                    inst.sync_info = mybir.SyncInfo(
                        on_wait=waits[-keep:], on_update=list(si.on_update)
                    )
                out.append(inst)
            blk.instructions[:] = out
    return n_split


def _build_program():
    import concourse.bass as bass
    import concourse.tile as tile
    import concourse.mybir as mybir

    F32 = mybir.dt.float32
    MMDT = mybir.dt.float32r if MM_MODE == "f32r" else mybir.dt.bfloat16
    AF = mybir.ActivationFunctionType
    ALU = mybir.AluOpType

    nc = bass.Bass()
    x0_d = nc.dram_tensor("x0", [2, 128, NEXT], MMDT, kind="ExternalInput")
    wc_d = nc.dram_tensor("wc", [L, 128, 3, 2, 512], MMDT, kind="ExternalInput")
    wo_d = nc.dram_tensor("wo", [L - 1, 128, 2, 256], MMDT, kind="ExternalInput")
    bc_d = nc.dram_tensor("bc", [128, L * 4], F32, kind="ExternalInput")
    bo_d = nc.dram_tensor("bo", [128, (L - 1) * 2], F32, kind="ExternalInput")
    mask_d = nc.dram_tensor("mask", [128, 1], F32, kind="ExternalInput")
    xout_d = nc.dram_tensor("xout", [2, 128, CH], F32, kind="ExternalOutput")
    skips_d = nc.dram_tensor("skips", [L, 2, 128, CH], F32, kind="ExternalOutput")

    def as_f32(ap):
        return ap.bitcast(F32) if MM_MODE == "f32r" else ap

    with tile.TileContext(nc) as tc:
        with (
            tc.tile_pool(name="xb", bufs=1) as xbp,
            tc.tile_pool(name="wc", bufs=2) as wcp,
            tc.tile_pool(name="wo", bufs=2) as wop,
            tc.tile_pool(name="const", bufs=1) as cstp,
            tc.tile_pool(name="th", bufs=3) as thp,
            tc.tile_pool(name="sg", bufs=3) as sgp,
            tc.tile_pool(name="h", bufs=4) as hp,
            tc.tile_pool(name="cps", bufs=6, space="PSUM") as cps,
            tc.tile_pool(name="ops", bufs=2, space="PSUM") as ops,
        ):
            xb = xbp.tile([128, 2, NEXT], MMDT, name="xb", tag="xb")
            bc_t = cstp.tile([128, L * 4], F32, name="bc", tag="bc")
            bo_t = cstp.tile([128, (L - 1) * 2], F32, name="bo", tag="bo")
            mk_t = cstp.tile([128, 1], F32, name="mk", tag="mk")
            for g in range(2):
                nc.sync.dma_start(xb[:, g, :], x0_d[g])
            nc.sync.dma_start(bc_t[:], bc_d[:])
            nc.sync.dma_start(bo_t[:], bo_d[:])
            nc.sync.dma_start(mk_t[:], mask_d[:])

            wc_t = wcp.tile([128, 3, 2, 512], MMDT, name="wc", tag="wc")
            nc.sync.dma_start(wc_t[:], wc_d[0])
            wo_t = wop.tile([128, 2, 256], MMDT, name="wo", tag="wo")
            nc.sync.dma_start(wo_t[:], wo_d[0])
            tc.strict_bb_all_engine_barrier()

            for l in range(L):
                d = DILATIONS[l]
                tiles = _layer_tiles(l)
                # prefetch next layer's weights
                wc_next = wo_next = None
                if l + 1 < L:
                    wc_next = wcp.tile([128, 3, 2, 512], MMDT, name="wc", tag="wc")
                    nc.sync.dma_start(wc_next[:], wc_d[l + 1])
                    if l + 1 < L - 1:
                        wo_next = wop.tile([128, 2, 256], MMDT, name="wo", tag="wo")
                        nc.sync.dma_start(wo_next[:], wo_d[l + 1])

                pending = None  # (t0, t1, h_tile) awaiting 1x1+residual

                def flush(pend):
                    t0, t1, h_t = pend
                    w = t1 - t0
                    halo = t0 < U0
                    if l < L - 1:
                        ps2 = [ops.tile([128, w], F32, name="op", tag="op") for _ in range(2)]
                        for o2 in range(2):
                            for g2 in range(2):
                                nc.tensor.matmul(
                                    ps2[o2][:],
                                    wo_t[:, g2, o2 * 128:(o2 + 1) * 128],
                                    h_t[:, g2, :],
                                    start=(g2 == 0),
                                    stop=(g2 == 1),
                                )
                        for o2 in range(2):
                            # x = (psum + b_out) + x  in one DVE op
                            nc.vector.scalar_tensor_tensor(
                                xb[:, o2, t0:t1],
                                ps2[o2][:],
                                bo_t[:, l * 2 + o2:l * 2 + o2 + 1],
                                xb[:, o2, t0:t1],
                                op0=ALU.add,
                                op1=ALU.add,
                            )
                    else:
                        for g2 in range(2):
                            nc.vector.tensor_add(
                                xb[:, g2, t0:t1], h_t[:, g2, :], xb[:, g2, t0:t1]
                            )
                    if halo:
                        for g2 in range(2):
                            nc.vector.tensor_scalar_mul(
                                xb[:, g2, t0:t1], xb[:, g2, t0:t1], mk_t[:]
                            )

                for (t0, t1) in tiles:
                    w = t1 - t0
                    cps_t = [cps.tile([128, w], F32, name="cp", tag="cp") for _ in range(4)]
                    for o in range(4):
                        first = True
                        for k in range(3):
                            sh = (2 - k) * d
                            for g in range(2):
                                nc.tensor.matmul(
                                    cps_t[o][:],
                                    wc_t[:, k, g, o * 128:(o + 1) * 128],
                                    xb[:, g, t0 - sh:t1 - sh],
                                    start=first,
                                    stop=(k == 2 and g == 1),
                                )
                                first = False
                    th_t = thp.tile([128, 2, w], F32, name="th", tag="th")
                    sg_t = sgp.tile([128, 2, w], F32, name="sg", tag="sg")
                    for o in range(2):
                        nc.scalar.activation(
                            th_t[:, o, :], cps_t[o][:], AF.Tanh,
                            bias=bc_t[:, l * 4 + o:l * 4 + o + 1],
                        )
                    for o in range(2):
                        nc.scalar.activation(
                            sg_t[:, o, :], cps_t[2 + o][:], AF.Sigmoid,
                            bias=bc_t[:, l * 4 + 2 + o:l * 4 + 3 + o],
                        )
                    h_t = hp.tile([128, 2, w], MMDT, name="h", tag="h")
                    for g2 in range(2):
                        nc.vector.tensor_mul(
                            h_t[:, g2, :], th_t[:, g2, :], sg_t[:, g2, :]
                        )
                    if t0 >= U0:  # interior: emit skip output
                        for g2 in range(2):
                            nc.sync.dma_start(
                                skips_d[l, g2][:, t0 - U0:t1 - U0],
                                as_f32(h_t[:, g2, :]),
                            )
                    if pending is not None:
                        flush(pending)
                    pending = (t0, t1, h_t)
                flush(pending)
                if wc_next is not None:
                    wc_t = wc_next
                if wo_next is not None:
                    wo_t = wo_next

            for g in range(2):
                nc.sync.dma_start(xout_d[g], as_f32(xb[:, g, U0:NEXT]))

    _legalize_waits(nc, keep=1)
    return nc


def _prep_inputs(input, weights_conv, biases_conv, weights_out, biases_out):
    if MM_MODE == "bf16":
        import ml_dtypes
        mdt = ml_dtypes.bfloat16
    else:
        mdt = np.float32

    wc = (
        weights_conv.transpose(0, 2, 3, 1)        # [L, cin, K, cout]
        .reshape(L, 2, 128, 3, 512)
        .transpose(0, 2, 3, 1, 4)                 # [L, 128, K, g, cout]
        .astype(mdt)
    )
    wo = (
        weights_out[: L - 1].transpose(0, 2, 1)   # [9, cin, cout]
        .reshape(L - 1, 2, 128, 256)
        .transpose(0, 2, 1, 3)                    # [9, 128, g, cout]
        .astype(mdt)
    )
    bc = np.ascontiguousarray(
        biases_conv.reshape(L, 4, 128).transpose(2, 0, 1).reshape(128, L * 4)
    ).astype(np.float32)
    bo = np.ascontiguousarray(
        biases_out[: L - 1].reshape(L - 1, 2, 128).transpose(2, 0, 1)
        .reshape(128, (L - 1) * 2)
    ).astype(np.float32)
    wc = np.ascontiguousarray(wc)
    wo = np.ascontiguousarray(wo)

    in_maps = []
    for c in range(8):
        b, half = c // 2, c % 2
        x0 = np.zeros((C, NEXT), np.float32)
        if half == 0:
            x0[:, U0:] = input[b, :, :CH]
        else:
            x0[:, :] = input[b, :, CH - U0:]
        mask = np.zeros((128, 1), np.float32) if half == 0 else np.ones(
            (128, 1), np.float32
        )
        in_maps.append(
            {
                "x0": np.ascontiguousarray(x0.reshape(2, 128, NEXT)).astype(mdt),
                "wc": wc,
                "wo": wo,
                "bc": bc,
                "bo": bo,
                "mask": mask,
            }
        )
    return in_maps


def _assemble(results):
    x = np.empty((B, C, T), np.float32)
    skips = np.empty((L, B, C, T), np.float32)
    for c, res in enumerate(results):
        b, half = c // 2, c % 2
        sl = slice(half * CH, (half + 1) * CH)
        x[b, :, sl] = res["xout"].reshape(C, CH)
        skips[:, b, :, sl] = res["skips"].reshape(L, C, CH)
    return x, skips


def kernel(input, weights_conv, biases_conv, weights_out, biases_out):
    from concourse.bass_utils import run_bass_kernel_spmd

    if "nc" not in _CACHE:
        _CACHE["nc"] = _build_program()
    nc = _CACHE["nc"]
    in_maps = _prep_inputs(
        np.asarray(input, np.float32),
        np.asarray(weights_conv, np.float32),
        np.asarray(biases_conv, np.float32),
        np.asarray(weights_out, np.float32),
        np.asarray(biases_out, np.float32),
    )
    r = run_bass_kernel_spmd(nc, in_maps, core_ids=list(range(8)))
    return _assemble(r.results)


# revision 8
# speedup vs baseline: 1.0095x; 1.0095x over previous
"""WaveNet-style dilated conv stack on 8 TRN2 NeuronCores.

Sharding: batch(4) x time(2). Each core gets one batch element and one
4096-sample half of the sequence plus a 2048-sample left halo window
(receptive field = 2046). Left-half cores get zeros in the halo plus a
0.0 mask that re-zeroes the halo region after every layer, which exactly
emulates causal zero padding; right-half cores carry real halo data and
mask 1.0 (redundant halo recompute, zero inter-core communication).

Per layer: the dilated conv is 24 accumulated 128x128x512 matmuls per
512-col tile, reading shifted views of a persistent in-SBUF x buffer
(no im2col). Gated tanh*sigmoid on ScalarE with the conv bias fused into
the activation; 1x1 conv = 4 matmuls; bias + residual fused into one
VectorE scalar_tensor_tensor; x updated in place (right-to-left tile
order makes that safe). Matmuls run in float32r (fp32 storage, relaxed
precision, full PE rate at N>=256).
"""

import sys
import numpy as np

sys.path.insert(0, "/opt/trn_rl_repo")

DILATIONS = [1, 2, 4, 8, 16, 32, 64, 128, 256, 512]
L = 10
C = 256
B = 4
T = 8192
CH = T // 2          # per-core chunk
U0 = 2048            # halo width (>= receptive field 2046)
NEXT = U0 + CH       # per-core x window
TW = 512             # tile width

MM_MODE = "f32r"

_CACHE = {}


def _h_after(l):
    """Receptive-field halo needed for the output of layer l."""
    return 2 * sum(DILATIONS[l + 1:])


def _layer_tiles(l):
    """(lo, hi) output-column tiles for layer l, right-to-left order."""
    a = U0 - _h_after(l)
    edges = [a]
    e = (a // TW + 1) * TW
    while e <= NEXT:
        edges.append(e)
        e += TW
    tiles = [(edges[i], edges[i + 1]) for i in range(len(edges) - 1)]
    return tiles[::-1]


def _legalize_waits(nc, keep=1):
    """This walrus build rejects instructions carrying more than ~1 sync
    wait. Move extra waits onto preceding same-engine NOPs, one each."""
    import concourse.mybir as mybir

    n_split = 0
    for f in nc.m.functions:
        for blk in f.blocks:
            out = []
            for inst in blk.instructions:
                si = inst.sync_info
                if si is not None and len(si.on_wait) > keep:
                    waits = list(si.on_wait)
                    for w in waits[:-keep]:
                        nop = mybir.InstNoOp(
                            name=nc.get_next_instruction_name(), ins=[], outs=[]
                        )
                        nop.engine = inst.engine
                        nop.sync_info = mybir.SyncInfo(on_wait=[w], on_update=[])
                        out.append(nop)
                        n_split += 1
                    inst.sync_info = mybir.SyncInfo(
                        on_wait=waits[-keep:], on_update=list(si.on_update)
                    )
                out.append(inst)
            blk.instructions[:] = out
    return n_split


def _build_program(reps=1, ablate="none"):
    import concourse.bass as bass
    import concourse.tile as tile
    import concourse.mybir as mybir

    F32 = mybir.dt.float32
    MMDT = mybir.dt.float32r if MM_MODE == "f32r" else mybir.dt.bfloat16
    AF = mybir.ActivationFunctionType
    ALU = mybir.AluOpType

    nc = bass.Bass()
    x0_d = nc.dram_tensor("x0", [2, 128, NEXT], MMDT, kind="ExternalInput")
    wc_d = nc.dram_tensor("wc", [L, 128, 3, 2, 512], MMDT, kind="ExternalInput")
    wo_d = nc.dram_tensor("wo", [L - 1, 128, 2, 256], MMDT, kind="ExternalInput")
    bc_d = nc.dram_tensor("bc", [128, L * 4], F32, kind="ExternalInput")
    bo_d = nc.dram_tensor("bo", [128, (L - 1) * 2], F32, kind="ExternalInput")
    mask_d = nc.dram_tensor("mask", [128, 1], F32, kind="ExternalInput")
    xout_d = nc.dram_tensor("xout", [2, 128, CH], F32, kind="ExternalOutput")
    skips_d = nc.dram_tensor("skips", [L, 2, 128, CH], F32, kind="ExternalOutput")

    def as_f32(ap):
        return ap.bitcast(F32) if MM_MODE == "f32r" else ap

    with tile.TileContext(nc) as tc:
        with (
            tc.tile_pool(name="xb", bufs=1) as xbp,
            tc.tile_pool(name="wc", bufs=2) as wcp,
            tc.tile_pool(name="wo", bufs=2) as wop,
            tc.tile_pool(name="const", bufs=1) as cstp,
            tc.tile_pool(name="th", bufs=3) as thp,
            tc.tile_pool(name="sg", bufs=3) as sgp,
            tc.tile_pool(name="h", bufs=4) as hp,
            tc.tile_pool(name="cps", bufs=6, space="PSUM") as cps,
            tc.tile_pool(name="ops", bufs=2, space="PSUM") as ops,
        ):
            for rep in range(reps):
                xb = xbp.tile([128, 2, NEXT], MMDT, name="xb", tag="xb")
                bc_t = cstp.tile([128, L * 4], F32, name="bc", tag="bc")
                bo_t = cstp.tile([128, (L - 1) * 2], F32, name="bo", tag="bo")
                mk_t = cstp.tile([128, 1], F32, name="mk", tag="mk")
                for g in range(2):
                    nc.sync.dma_start(xb[:, g, :], x0_d[g])
                nc.sync.dma_start(bc_t[:], bc_d[:])
                nc.sync.dma_start(bo_t[:], bo_d[:])
                nc.sync.dma_start(mk_t[:], mask_d[:])

                wc_t = wcp.tile([128, 3, 2, 512], MMDT, name="wc", tag="wc")
                nc.sync.dma_start(wc_t[:], wc_d[0])
                wo_t = wop.tile([128, 2, 256], MMDT, name="wo", tag="wo")
                nc.sync.dma_start(wo_t[:], wo_d[0])
                tc.strict_bb_all_engine_barrier()

                for l in range(L):
                    d = DILATIONS[l]
                    tiles = _layer_tiles(l)
                    wc_next = wo_next = None
                    if l + 1 < L:
                        wc_next = wcp.tile(
                            [128, 3, 2, 512], MMDT, name="wc", tag="wc"
                        )
                        nc.sync.dma_start(wc_next[:], wc_d[l + 1])
                        if l + 1 < L - 1:
                            wo_next = wop.tile(
                                [128, 2, 256], MMDT, name="wo", tag="wo"
                            )
                            nc.sync.dma_start(wo_next[:], wo_d[l + 1])

                    pending = None  # (t0, t1, h_tile) awaiting 1x1+residual

                    def flush(pend, l=l, wo_t=wo_t):
                        t0, t1, h_t = pend
                        w = t1 - t0
                        halo = t0 < U0
                        if l < L - 1:
                            ps2 = [
                                ops.tile([128, w], F32, name="op", tag="op")
                                for _ in range(2)
                            ]
                            for o2 in range(2):
                                for g2 in range(2):
                                    nc.tensor.matmul(
                                        ps2[o2][:],
                                        wo_t[:, g2, o2 * 128:(o2 + 1) * 128],
                                        h_t[:, g2, :],
                                        start=(g2 == 0),
                                        stop=(g2 == 1),
                                    )
                            for o2 in range(2):
                                # x = (psum + b_out) + x in one DVE op
                                nc.vector.scalar_tensor_tensor(
                                    xb[:, o2, t0:t1],
                                    ps2[o2][:],
                                    bo_t[:, l * 2 + o2:l * 2 + o2 + 1],
                                    xb[:, o2, t0:t1],
                                    op0=ALU.add,
                                    op1=ALU.add,
                                )
                        else:
                            for g2 in range(2):
                                nc.vector.tensor_add(
                                    xb[:, g2, t0:t1], h_t[:, g2, :],
                                    xb[:, g2, t0:t1],
                                )
                        if halo:
                            for g2 in range(2):
                                nc.vector.tensor_scalar_mul(
                                    xb[:, g2, t0:t1], xb[:, g2, t0:t1], mk_t[:]
                                )

                    for (t0, t1) in tiles:
                        w = t1 - t0
                        cps_t = [
                            cps.tile([128, w], F32, name="cp", tag="cp")
                            for _ in range(4)
                        ]
                        for o in range(4):
                            first = True
                            for k in range(3):
                                sh = (2 - k) * d
                                for g in range(2):
                                    nc.tensor.matmul(
                                        cps_t[o][:],
                                        wc_t[:, k, g, o * 128:(o + 1) * 128],
                                        xb[:, g, t0 - sh:t1 - sh],
                                        start=first,
                                        stop=(k == 2 and g == 1),
                                    )
                                    first = False
                        if ablate == "pe_only":
                            continue
                        th_t = thp.tile([128, 2, w], F32, name="th", tag="th")
                        sg_t = sgp.tile([128, 2, w], F32, name="sg", tag="sg")
                        for o in range(2):
                            nc.scalar.activation(
                                th_t[:, o, :], cps_t[o][:], AF.Tanh,
                                bias=bc_t[:, l * 4 + o:l * 4 + o + 1],
                            )
                        for o in range(2):
                            nc.scalar.activation(
                                sg_t[:, o, :], cps_t[2 + o][:], AF.Sigmoid,
                                bias=bc_t[:, l * 4 + 2 + o:l * 4 + 3 + o],
                            )
                        h_t = hp.tile([128, 2, w], MMDT, name="h", tag="h")
                        for g2 in range(2):
                            nc.vector.tensor_mul(
                                h_t[:, g2, :], th_t[:, g2, :], sg_t[:, g2, :]
                            )
                        if t0 >= U0:  # interior: emit skip output
                            for g2 in range(2):
                                nc.sync.dma_start(
                                    skips_d[l, g2][:, t0 - U0:t1 - U0],
                                    as_f32(h_t[:, g2, :]),
                                )
                        if pending is not None:
                            flush(pending)
                        pending = (t0, t1, h_t)
                    if pending is not None:
                        flush(pending)
                    if wc_next is not None:
                        wc_t = wc_next
                    if wo_next is not None:
                        wo_t = wo_next

                for g in range(2):
                    nc.sync.dma_start(xout_d[g], as_f32(xb[:, g, U0:NEXT]))

    _legalize_waits(nc, keep=1)
    return nc


def _prep_inputs(input, weights_conv, biases_conv, weights_out, biases_out):
    if MM_MODE == "bf16":
        import ml_dtypes
        mdt = ml_dtypes.bfloat16
    else:
        mdt = np.float32

    wc = (
        weights_conv.transpose(0, 2, 3, 1)        # [L, cin, K, cout]
        .reshape(L, 2, 128, 3, 512)
        .transpose(0, 2, 3, 1, 4)                 # [L, 128, K, g, cout]
        .astype(mdt)
    )
    wo = (
        weights_out[: L - 1].transpose(0, 2, 1)   # [9, cin, cout]
        .reshape(L - 1, 2, 128, 256)
        .transpose(0, 2, 1, 3)                    # [9, 128, g, cout]
        .astype(mdt)
    )
    bc = np.ascontiguousarray(
        biases_conv.reshape(L, 4, 128).transpose(2, 0, 1).reshape(128, L * 4)
    ).astype(np.float32)
    bo = np.ascontiguousarray(
        biases_out[: L - 1].reshape(L - 1, 2, 128).transpose(2, 0, 1)
        .reshape(128, (L - 1) * 2)
    ).astype(np.float32)
    wc = np.ascontiguousarray(wc)
    wo = np.ascontiguousarray(wo)

    in_maps = []
    for c in range(8):
        b, half = c // 2, c % 2
        x0 = np.zeros((C, NEXT), np.float32)
        if half == 0:
            x0[:, U0:] = input[b, :, :CH]
        else:
            x0[:, :] = input[b, :, CH - U0:]
        mask = (
            np.zeros((128, 1), np.float32)
            if half == 0
            else np.ones((128, 1), np.float32)
        )
        in_maps.append(
            {
                "x0": np.ascontiguousarray(x0.reshape(2, 128, NEXT)).astype(mdt),
                "wc": wc,
                "wo": wo,
                "bc": bc,
                "bo": bo,
                "mask": mask,
            }
        )
    return in_maps


def _assemble(results):
    x = np.empty((B, C, T), np.float32)
    skips = np.empty((L, B, C, T), np.float32)
    for c, res in enumerate(results):
        b, half = c // 2, c % 2
        sl = slice(half * CH, (half + 1) * CH)
        x[b, :, sl] = res["xout"].reshape(C, CH)
        skips[:, b, :, sl] = res["skips"].reshape(L, C, CH)
    return x, skips


def kernel(input, weights_conv, biases_conv, weights_out, biases_out):
    from concourse.bass_utils import run_bass_kernel_spmd

    if "nc" not in _CACHE:
        _CACHE["nc"] = _build_program()
    nc = _CACHE["nc"]
    in_maps = _prep_inputs(
        np.asarray(input, np.float32),
        np.asarray(weights_conv, np.float32),
        np.asarray(biases_conv, np.float32),
        np.asarray(weights_out, np.float32),
        np.asarray(biases_out, np.float32),
    )
    r = run_bass_kernel_spmd(nc, in_maps, core_ids=list(range(8)))
    return _assemble(r.results)
